# revision 40
# baseline (speedup 1.0000x reference)
"""Multi-head attention forward (B=8, N=1024, C=768, H=12, D=64) on 8 TRN2 NeuronCores.

Strategy: pure data-parallel over batch (batch 8 == 8 cores, no collectives).
Each core computes one full batch element. Host scatters inputs / gathers outputs.

Per-core kernel (bf16 TensorE compute, f32 PSUM accumulation):
  xT   = transpose(x)                          PE transposes, [C, N]
  qkT  = W_qkv[:, :2C].T @ xT (+b)             [2C, N]  (q,k transposed: head dim on partitions)
  V    = x @ W_qkv[:, 2C:] (+b)                [N, C]   (natural: k-token on partitions)
  PT_h = exp(SCALE * kT_h.T @ qT_h)            [N_k, N_q] per head (scores transposed; no
                                               max-subtraction needed: scores ~ N(0,1))
  av_h = [v_h | 1].T @ PT_h                    [65, N_q]: rows 0-63 unnormalized out^T,
                                               row 64 = softmax denominator
  aoT_h = av_h[0:64] * bcast(1/av_h[64])       attn_out transposed [C, N]
  out  = aoT.T @ W_out + b                     [N, C]

Performance notes (309us baseline -> 217us):
 - inputs are pre-cast to bf16 on the host and loaded with HWDGE
   (sync.dma_start): kills the serial ~650ns-per-DMA gpsimd issue chain
   that gated startup, halves input DMA bytes. First exp at ~24us
   (was ~38us).
 - score matmuls have K=64 (head dim): bass auto-emits them row-tiled
   64x128, so the two parities of a head pair (stationary at partitions
   0:64 / 64:128) execute on independent PE tiles T0/T8. Emitting both
   parities back-to-back per (qc, kc) lets the tiles overlap.
 - AV runs qc-outer over 1-bank [65, 512] PSUM tiles (2 tags x bufs=2)
   so each qc-half normalizes while the other accumulates; the final
   pair runs kc-outer so the post-last-exp tail is only the last kc.
 - softmax denominators: PSUM row staged to SBUF with a regular
   tensor_copy, then reciprocal_approx_fast (~5x faster than
   InstReciprocal; 18 bits, plenty above bf16). The copy is mandatory
   for correctness, not just speed: custom-DVE ops lose their
   PSUM-read wait in the finalize passes and race the accumulating
   matmuls on HW (CoreSim orders by program, so only HW corrupts).
 - V / out-proj biases are DVE tensor_adds against partition-broadcast
   bias tiles instead of K=1 PE matmuls.
 - exp (ScalarE, ~106us busy) paces phase 2 at ~2.2us per kc step;
   phase 1 is PE-bound (qkv projection work interleaved with pairs
   0-1's scores at ratio 14), phase 2 interleaves pairs 2-5's scores
   with all AV work at ratio 6.
 - PSUM budget (8 banks): phase 1 = scores 4 + proj 4; phase 2 =
   scores 4 + AV 4; tail = out-proj 4x2-bank tiles in two waves whose
   jc0-4 partials overlap the final normalize chain.
"""
import sys

sys.path.insert(0, "/opt/trn_rl_repo")

from contextlib import ExitStack

import numpy as np

import concourse.bass as bass
import concourse.bacc as bacc
import concourse.tile as tile
from concourse import masks, mybir

_SENTINEL = object()
F32 = mybir.dt.float32
BF = mybir.dt.bfloat16
AF = mybir.ActivationFunctionType

B, N, C, H, D = 8, 1024, 768, 12, 64
SCALE = D ** -0.5
NCORES = 8
NT = N // 128      # 8 token chunks
NCIN = C // 128    # 6 input-channel chunks
NPAIR = H // 2     # 6 head pairs

_DEBUG = False


def build():
    nc = bacc.Bacc()
    if _DEBUG:
        dbg = {
            nm: nc.declare_dram_parameter(nm, shape, F32, isOutput=True)
            for nm, shape in [
                ("dbg_xT", [128, N]), ("dbg_q", [128, N]), ("dbg_k", [128, N]),
                ("dbg_v", [128, H * (D + 1)]), ("dbg_pt", [128, N]),
                ("dbg_ao", [128, N]),
            ]
        }
    x_ext = nc.declare_dram_parameter("x_bf", [N, C], BF, isOutput=False)
    wq_ext = nc.declare_dram_parameter("W_qkv_bf", [C, 3 * C], BF, isOutput=False)
    bq_ext = nc.declare_dram_parameter("b_qkv", [3 * C], F32, isOutput=False)
    wo_ext = nc.declare_dram_parameter("W_out_bf", [C, C], BF, isOutput=False)
    bo_ext = nc.declare_dram_parameter("b_out", [C], F32, isOutput=False)
    out_ext = nc.declare_dram_parameter("out", [N, C], F32, isOutput=True)

    with ExitStack() as ctx:
        tc = ctx.enter_context(tile.TileContext(nc, pool_alloc_mode="queue"))
        persist = ctx.enter_context(tc.tile_pool(name="persist", bufs=1))

        # warm the ScalarE exp table during startup dead time: the
        # ~1.3us ACT_TABLE_LOAD otherwise sits at the head of the
        # critical exp chain at its first use
        warm = persist.tile([1, 1], F32, tag="warm")
        nc.vector.memset(warm[:], 0.0)
        warm2 = persist.tile([1, 1], F32, tag="warm2")
        nc.scalar.activation(warm2[:], warm[:], AF.Exp)

        # x loads first (HWDGE, bf16 pre-cast on host): they gate the PE
        # transposes that open the kernel. Pool (not persist) so the ring
        # reclaims the space once the transposes have consumed them.
        xbf_stack = ExitStack()
        xbf_pool = xbf_stack.enter_context(tc.tile_pool(name="xbf", bufs=1))
        xbf = []
        for t in range(NT):
            xb = xbf_pool.tile([128, C], BF, tag=f"xb{t}", name=f"xb{t}")
            nc.sync.dma_start(xb[:], x_ext[t * 128:(t + 1) * 128, :])
            xbf.append(xb)

        # identity for PE transposes (gpsimd-built, DVE-fenced)
        ident_g = persist.tile([128, 128], BF, tag="identg")
        masks.make_identity(nc, ident_g[:])
        ident = persist.tile([128, 128], BF, tag="ident")
        nc.vector.tensor_copy(ident[:], ident_g[:])

        # weights: HWDGE loads of host-pre-cast bf16
        wq_sb = []
        for j in range(NCIN):
            w = persist.tile([128, 3 * C], BF, tag=f"wq{j}", name=f"wq{j}")
            nc.sync.dma_start(w[:], wq_ext[j * 128:(j + 1) * 128, :])
            wq_sb.append(w)
        wo_sb = [
            persist.tile([128, C], BF, tag=f"wo{j}", name=f"wo{j}")
            for j in range(NCIN)
        ]

        # biases: per-cout column layout for q/k (f32, read only by DVE) and
        # full-partition broadcast tiles for the V / out-proj DVE adds
        bqT = persist.tile([128, 18], F32, tag="bqT")
        nc.sync.dma_start(bqT[:], bq_ext[:].rearrange("(j p) -> p j", p=128))
        bv_row = persist.tile([1, H, D + 1], F32, tag="bvrow")
        nc.vector.memset(bv_row[:], 0.0)
        nc.sync.dma_start(
            bv_row[:, :, 0:D],
            bq_ext[2 * C:3 * C].rearrange("(a h d) -> a h d", a=1, h=H),
        )
        bo_row = persist.tile([1, C], F32, tag="borow")
        nc.sync.dma_start(bo_row[:], bo_ext[:].rearrange("(a b) -> a b", a=1))
        bv_bc = persist.tile([128, H, D + 1], F32, tag="bvbc")
        nc.gpsimd.partition_broadcast(
            bv_bc[:].rearrange("p h d -> p (h d)"),
            bv_row[:].rearrange("a h d -> a (h d)"),
        )
        bo_bc = persist.tile([128, C], F32, tag="bobc")
        nc.gpsimd.partition_broadcast(bo_bc[:], bo_row[:])

        xT = [persist.tile([128, N], BF, tag=f"xT{j}", name=f"xT{j}") for j in range(NCIN)]
        qk_sb = [persist.tile([128, N], BF, tag=f"qk{j}", name=f"qk{j}") for j in range(2 * NCIN)]
        vaug = [persist.tile([128, H, D + 1], BF, tag=f"v{t}", name=f"v{t}") for t in range(NT)]
        ao = [persist.tile([128, N], BF, tag=f"ao{j}", name=f"ao{j}") for j in range(NCIN)]

        # ---- x transpose: PE transpose of bf16 chunks, DVE copies into xT ----
        with tc.tile_pool(name="pxt", bufs=4, space="PSUM") as pxt_pool:
            for t in range(NT):
                for j in range(NCIN):
                    pxt = pxt_pool.tile([128, 128], BF, tag="pxt")
                    nc.tensor.transpose(
                        pxt[:], xbf[t][:, j * 128:(j + 1) * 128], ident[:]
                    )
                    nc.vector.tensor_copy(
                        xT[j][:, t * 128:(t + 1) * 128], pxt[:]
                    )
        xbf_stack.close()

        # ---- x transpose: PE transpose of bf16 chunks, DVE copies into xT ----
        with tc.tile_pool(name="pxt", bufs=4, space="PSUM") as pxt_pool:
            for t in range(NT):
                for j in range(NCIN):
                    pxt = pxt_pool.tile([128, 128], BF, tag="pxt")
                    nc.tensor.transpose(
                        pxt[:], xbf[t][:, j * 128:(j + 1) * 128], ident[:]
                    )
                    nc.vector.tensor_copy(
                        xT[j][:, t * 128:(t + 1) * 128], pxt[:]
                    )

        # ---- attention pipeline pools ----
        ps_stack = ExitStack()
        ps_pool = ps_stack.enter_context(tc.tile_pool(name="ps", bufs=2, space="PSUM"))
        av_stack = ExitStack()
        pt_pool = ctx.enter_context(tc.tile_pool(name="pt", bufs=34))
        rec_pool = ctx.enter_context(tc.tile_pool(name="rec", bufs=2))
        pbs_pool = ctx.enter_context(tc.tile_pool(name="pbs", bufs=2))
        # scratch slots are single-use: a reused slot would add the shuffle
        # DMA's queue semaphore to the normalize mul's wait list
        scr_pool = ctx.enter_context(tc.tile_pool(name="scr", bufs=2))
        out_pool = ctx.enter_context(tc.tile_pool(name="osb", bufs=2))

        proj_ctx = ExitStack()
        proj_pool = proj_ctx.enter_context(
            tc.tile_pool(name="proj", bufs=2, space="PSUM")
        )

        def _proj_tile():
            return proj_pool.tile([128, N], F32, tag="proj", name="proj")

        def gen_qk_chunk(jout):
            """q/k projection chunk jout (0-5: q, 6-11: k), output transposed.
            Yields after each PE matmul so the driver can interleave."""
            pq = _proj_tile()
            for qc in range(2):
                for jc in range(NCIN):
                    nc.tensor.matmul(
                        pq[:, qc * 512:(qc + 1) * 512],
                        wq_sb[jc][:, jout * 128:(jout + 1) * 128],
                        xT[jc][:, qc * 512:(qc + 1) * 512],
                        start=(jc == 0),
                        stop=(jc == NCIN - 1),
                    )
                    yield
            # PSUM -> SBUF bf16 with per-partition (per-cout) bias add
            nc.vector.tensor_scalar_add(
                qk_sb[jout][:], pq[:], bqT[:, jout:jout + 1]
            )

        def gen_v_chunk(t):
            """V projection for token chunk t, natural layout, into vaug.
            Bias is applied by the DVE drain (no K=1 PE matmul)."""
            pv = _proj_tile()
            for n0, n1 in ((0, 512), (512, 768)):
                for jc in range(NCIN):
                    nc.tensor.matmul(
                        pv[:, n0:n1],
                        xT[jc][:, t * 128:(t + 1) * 128],
                        wq_sb[jc][:, 2 * C + n0:2 * C + n1],
                        start=(jc == 0),
                        stop=(jc == NCIN - 1),
                    )
                    yield
            nc.vector.tensor_add(
                vaug[t][:, :, 0:D],
                pv[:, 0:C].rearrange("p (h d) -> p h d", h=H),
                bv_bc[:, :, 0:D],
            )
            nc.vector.memset(vaug[t][:, :, D:D + 1], 1.0)

        def gen_scores(pj, pts):
            """scores + exp for head pair pj; fills pts[par][kc].
            Both parities are emitted back-to-back per (qc, kc) so the two
            row-tiles (stationary at partitions 0:64 / 64:128) overlap.
            Yields once per kc (4 matmuls + 2 exps)."""
            for kc in range(NT):
                pt0 = pt_pool.tile([128, N], BF, tag="pt")
                pt1 = pt_pool.tile([128, N], BF, tag="pt")
                ps0 = ps_pool.tile([128, N], F32, tag="ps")
                ps1 = ps_pool.tile([128, N], F32, tag="ps")
                for qc in range(2):
                    for par, ps in ((0, ps0), (1, ps1)):
                        base = par * 64
                        nc.tensor.matmul(
                            ps[:, qc * 512:(qc + 1) * 512],
                            qk_sb[NCIN + pj][base:base + 64,
                                             kc * 128:(kc + 1) * 128],
                            qk_sb[pj][base:base + 64, qc * 512:(qc + 1) * 512],
                            start=True,
                            stop=True,
                        )
                nc.scalar.activation(pt0[:], ps0[:], AF.Exp, scale=SCALE)
                nc.scalar.activation(pt1[:], ps1[:], AF.Exp, scale=SCALE)
                pts[0].append(pt0)
                pts[1].append(pt1)
                yield

        av_pool = None

        def _av_norm(pj, av, par, sl, scr):
            # stage the denominator row through SBUF with a regular copy:
            # custom-DVE ops lose their PSUM-read wait in the finalize
            # passes (read-before-matmul-stop on HW; sim can't see it),
            # and DVE in-queue ordering makes copy->recip safe.
            den = rec_pool.tile([1, 512], F32, tag="den", name="den")
            nc.vector.tensor_copy(den[:], av[64:65, :])
            rec = rec_pool.tile([1, 512], F32, tag="rec", name="rec")
            nc.vector.reciprocal_approx_fast(rec[:], den[:])
            pb = pbs_pool.tile([64, 512], F32, tag="pbs", name="pb")
            nc.gpsimd.partition_broadcast(pb[:], rec[:])
            if par == 0:
                nc.vector.tensor_mul(ao[pj][0:64, sl], av[0:64, :], pb[:])
            else:
                nc.vector.tensor_mul(scr[:, sl], av[0:64, :], pb[:])

        def gen_av(pj, pts):
            """AV + normalization for pair pj, qc-outer over pairs of 1-bank
            PSUM tiles: each qc-half normalizes while the other half
            accumulates, so the reciprocal/broadcast/mul latency hides.
            Even head -> ao rows 0:64 directly; odd head -> scratch, DMA
            shuffle to rows 64:128. Yields after each PE matmul."""
            scr = scr_pool.tile([64, N], BF, tag="scr")
            for qc in range(2):
                sl = slice(qc * 512, (qc + 1) * 512)
                av = [
                    av_pool.tile([65, 512], F32, tag=f"av{par}", name=f"av{par}")
                    for par in range(2)
                ]
                for kc in range(NT):
                    for par in range(2):
                        nc.tensor.matmul(
                            av[par][:],
                            vaug[kc][:, 2 * pj + par, :],
                            pts[par][kc][:, sl],
                            start=(kc == 0),
                            stop=(kc == NT - 1),
                        )
                        yield
                for par in range(2):
                    _av_norm(pj, av[par], par, sl, scr)
            nc.sync.dma_start(ao[pj][64:128, :], scr[:])

        def gen_av_kc_outer(pj, pts):
            """kc-outer AV for the final pair: consumes each kc's exp as it
            is emitted (all four PSUM tiles open), so the post-last-exp
            tail is only the last kc's matmuls plus normalization."""
            scr = scr_pool.tile([64, N], BF, tag="scr")
            av = [
                [
                    av_pool.tile([65, 512], F32, tag=f"av{par}", name=f"av{par}")
                    for par in range(2)
                ]
                for qc in range(2)
            ]
            for kc in range(NT):
                for qc in range(2):
                    sl = slice(qc * 512, (qc + 1) * 512)
                    for par in range(2):
                        nc.tensor.matmul(
                            av[qc][par][:],
                            vaug[kc][:, 2 * pj + par, :],
                            pts[par][kc][:, sl],
                            start=(kc == 0),
                            stop=(kc == NT - 1),
                        )
                        yield
            # phased normalize: DVE copies+recips first, then gpsimd
            # broadcasts, then muls — so the DVE never head-of-line
            # blocks on a broadcast, and the exposed post-exp chain is
            # ~4us shorter than four serial copy/recip/bcast/mul chains.
            recs = {}
            for qc in range(2):
                for par in range(2):
                    den = rec_pool.tile([1, 512], F32, tag="den", name="den")
                    nc.vector.tensor_copy(den[:], av[qc][par][64:65, :])
                    rec = rec_pool.tile([1, 512], F32, tag="rec", name="rec")
                    nc.vector.reciprocal_approx_fast(rec[:], den[:])
                    recs[qc, par] = rec
            pbs = {}
            for qc in range(2):
                for par in range(2):
                    pb = pbs_pool.tile([64, 512], F32, tag="pbs", name="pb")
                    nc.gpsimd.partition_broadcast(pb[:], recs[qc, par][:])
                    pbs[qc, par] = pb
            for qc in range(2):
                sl = slice(qc * 512, (qc + 1) * 512)
                nc.vector.tensor_mul(
                    ao[pj][0:64, sl], av[qc][0][0:64, :], pbs[qc, 0][:]
                )
                nc.vector.tensor_mul(scr[:, sl], av[qc][1][0:64, :], pbs[qc, 1][:])
                nc.sync.dma_start(ao[pj][64:128, sl], scr[:, sl])

        def drive(primary, filler, ratio):
            """Alternate: 1 primary step then `ratio` filler steps; drain
            primary; leftover filler is left for the caller."""
            for _ in primary:
                for _ in range(ratio):
                    if next(filler, _SENTINEL) is _SENTINEL:
                        break

        def drain(g):
            for _ in g:
                pass

        def chain(*gens):
            for g in gens:
                yield from g

        # ---- emission schedule ----
        # Phase 1 (PE-bound): all qkv projection work as filler between
        # pairs 0-1's score steps (14 matmuls ≈ 3.2us per step).
        # Phase 2 (exp-paced): pairs 2-5's scores every ~2.2us with all
        # AV work as filler (6 yields per step).
        drain(gen_qk_chunk(0))
        drain(gen_qk_chunk(NCIN + 0))
        pts_all = {pj: [[], []] for pj in range(NPAIR)}
        early_scores = chain(
            gen_scores(0, pts_all[0]), gen_scores(1, pts_all[1])
        )
        filler = chain(
            gen_qk_chunk(1), gen_qk_chunk(NCIN + 1),
            *[gen_v_chunk(t) for t in range(NT)],
            gen_qk_chunk(2), gen_qk_chunk(NCIN + 2),
            gen_qk_chunk(3), gen_qk_chunk(NCIN + 3),
            gen_qk_chunk(4), gen_qk_chunk(NCIN + 4),
            gen_qk_chunk(5), gen_qk_chunk(NCIN + 5),
        )
        drive(early_scores, filler, 14)
        drain(filler)
        proj_ctx.close()
        # W_out is only needed by the out-projection tail: issuing its DMA
        # here keeps the startup bandwidth for x and W_qkv
        for j in range(NCIN):
            nc.sync.dma_start(wo_sb[j][:], wo_ext[j * 128:(j + 1) * 128, :])
        if _DEBUG:
            nc.gpsimd.dma_start(dbg["dbg_xT"][:], xT[0][:])
            nc.gpsimd.dma_start(dbg["dbg_q"][:], qk_sb[0][:])
            nc.gpsimd.dma_start(dbg["dbg_k"][:], qk_sb[NCIN][:])
            nc.gpsimd.dma_start(
                dbg["dbg_v"][:], vaug[0][:].rearrange("p h d -> p (h d)")
            )
            nc.gpsimd.dma_start(dbg["dbg_pt"][:], pts_all[0][0][0][:])

        av_pool = av_stack.enter_context(
            tc.tile_pool(name="av", bufs=2, space="PSUM")
        )

        late_scores = chain(
            gen_scores(2, pts_all[2]), gen_scores(3, pts_all[3]),
            gen_scores(4, pts_all[4]), gen_scores(5, pts_all[5]),
        )
        av_chain = chain(
            *[gen_av(pj, pts_all[pj]) for pj in range(NPAIR - 1)],
            gen_av_kc_outer(NPAIR - 1, pts_all[NPAIR - 1]),
        )
        drive(late_scores, av_chain, 6)
        drain(av_chain)
        av_stack.close()
        ps_stack.close()
        if _DEBUG:
            nc.gpsimd.dma_start(dbg["dbg_ao"][:], ao[0][:])

        # ---- output projection (bias via DVE add on the PSUM drain) ----
        # Two waves of 4 token chunks; within a wave all jc 0-4 partials
        # are emitted first so they overlap the final pair's normalize
        # chain, and only the jc=5 closers wait on ao[5].
        pf_pool = ctx.enter_context(tc.tile_pool(name="pf", bufs=4, space="PSUM"))
        for wave in range(2):
            ts = range(wave * 4, wave * 4 + 4)
            pfs = {}
            for t in ts:
                pfs[t] = pf_pool.tile([128, N], F32, tag="pf", name="pf")
                for n0, n1 in ((0, 512), (512, 768)):
                    for jc in range(NCIN - 1):
                        nc.tensor.matmul(
                            pfs[t][:, n0:n1],
                            ao[jc][:, t * 128:(t + 1) * 128],
                            wo_sb[jc][:, n0:n1],
                            start=(jc == 0),
                            stop=False,
                        )
            for t in ts:
                for n0, n1 in ((0, 512), (512, 768)):
                    nc.tensor.matmul(
                        pfs[t][:, n0:n1],
                        ao[NCIN - 1][:, t * 128:(t + 1) * 128],
                        wo_sb[NCIN - 1][:, n0:n1],
                        start=False,
                        stop=True,
                    )
                osb = out_pool.tile([128, C], F32, tag="osb")
                nc.vector.tensor_add(osb[:], pfs[t][:, 0:C], bo_bc[:])
                nc.sync.dma_start(out_ext[t * 128:(t + 1) * 128, :], osb[:])

    nc.finalize()
    return nc


_NC = None


def _get_nc():
    global _NC
    if _NC is None:
        _NC = build()
    return _NC


def _run(inputs, trace=False, **kw):
    from concourse.bass_utils import run_bass_kernel_spmd

    import ml_dtypes

    nc = _get_nc()
    bf = ml_dtypes.bfloat16
    x = np.ascontiguousarray(np.asarray(inputs["x"], dtype=np.float32).astype(bf))
    shared = {
        "W_qkv_bf": np.ascontiguousarray(
            np.asarray(inputs["W_qkv"], np.float32).astype(bf)),
        "b_qkv": np.ascontiguousarray(np.asarray(inputs["b_qkv"], np.float32)),
        "W_out_bf": np.ascontiguousarray(
            np.asarray(inputs["W_out"], np.float32).astype(bf)),
        "b_out": np.ascontiguousarray(np.asarray(inputs["b_out"], np.float32)),
    }
    in_maps = [dict(shared, x_bf=x[c]) for c in range(NCORES)]
    res = run_bass_kernel_spmd(
        nc, in_maps, core_ids=list(range(NCORES)), trace=trace, **kw
    )
    out = np.stack([res.results[c]["out"] for c in range(NCORES)], axis=0)
    return out.astype(np.float32), res


def kernel(**inputs):
    out, _ = _run(inputs, trace=False)
    return out


# revision 42
# speedup vs baseline: 1.0069x; 1.0069x over previous
"""Multi-head attention forward (B=8, N=1024, C=768, H=12, D=64) on 8 TRN2 NeuronCores.

Strategy: pure data-parallel over batch (batch 8 == 8 cores, no collectives).
Each core computes one full batch element. Host scatters inputs / gathers outputs.

Per-core kernel (bf16 TensorE compute, f32 PSUM accumulation):
  xT   = transpose(x)                          PE transposes, [C, N]
  qkT  = W_qkv[:, :2C].T @ xT (+b)             [2C, N]  (q,k transposed: head dim on partitions)
  V    = x @ W_qkv[:, 2C:] (+b)                [N, C]   (natural: k-token on partitions)
  PT_h = exp(SCALE * kT_h.T @ qT_h)            [N_k, N_q] per head (scores transposed; no
                                               max-subtraction needed: scores ~ N(0,1))
  av_h = [v_h | 1].T @ PT_h                    [65, N_q]: rows 0-63 unnormalized out^T,
                                               row 64 = softmax denominator
  aoT_h = av_h[0:64] * bcast(1/av_h[64])       attn_out transposed [C, N]
  out  = aoT.T @ W_out + b                     [N, C]

Performance notes (309us baseline -> 217us):
 - inputs are pre-cast to bf16 on the host and loaded with HWDGE
   (sync.dma_start): kills the serial ~650ns-per-DMA gpsimd issue chain
   that gated startup, halves input DMA bytes. First exp at ~24us
   (was ~38us).
 - score matmuls have K=64 (head dim): bass auto-emits them row-tiled
   64x128, so the two parities of a head pair (stationary at partitions
   0:64 / 64:128) execute on independent PE tiles T0/T8. Emitting both
   parities back-to-back per (qc, kc) lets the tiles overlap.
 - AV runs qc-outer over 1-bank [65, 512] PSUM tiles (2 tags x bufs=2)
   so each qc-half normalizes while the other accumulates; the final
   pair runs kc-outer so the post-last-exp tail is only the last kc.
 - softmax denominators: PSUM row staged to SBUF with a regular
   tensor_copy, then reciprocal_approx_fast (~5x faster than
   InstReciprocal; 18 bits, plenty above bf16). The copy is mandatory
   for correctness, not just speed: custom-DVE ops lose their
   PSUM-read wait in the finalize passes and race the accumulating
   matmuls on HW (CoreSim orders by program, so only HW corrupts).
 - V / out-proj biases are DVE tensor_adds against partition-broadcast
   bias tiles instead of K=1 PE matmuls.
 - exp (ScalarE, ~106us busy) paces phase 2 at ~2.2us per kc step;
   phase 1 is PE-bound (qkv projection work interleaved with pairs
   0-1's scores at ratio 14), phase 2 interleaves pairs 2-5's scores
   with all AV work at ratio 6.
 - PSUM budget (8 banks): phase 1 = scores 4 + proj 4; phase 2 =
   scores 4 + AV 4; tail = out-proj 4x2-bank tiles in two waves whose
   jc0-4 partials overlap the final normalize chain.
"""
import sys

sys.path.insert(0, "/opt/trn_rl_repo")

from contextlib import ExitStack

import numpy as np

import concourse.bass as bass
import concourse.bacc as bacc
import concourse.tile as tile
from concourse import masks, mybir

_SENTINEL = object()
F32 = mybir.dt.float32
BF = mybir.dt.bfloat16
AF = mybir.ActivationFunctionType

B, N, C, H, D = 8, 1024, 768, 12, 64
SCALE = D ** -0.5
NCORES = 8
NT = N // 128      # 8 token chunks
NCIN = C // 128    # 6 input-channel chunks
NPAIR = H // 2     # 6 head pairs

_DEBUG = False


def build():
    nc = bacc.Bacc()
    if _DEBUG:
        dbg = {
            nm: nc.declare_dram_parameter(nm, shape, F32, isOutput=True)
            for nm, shape in [
                ("dbg_xT", [128, N]), ("dbg_q", [128, N]), ("dbg_k", [128, N]),
                ("dbg_v", [128, H * (D + 1)]), ("dbg_pt", [128, N]),
                ("dbg_ao", [128, N]),
            ]
        }
    x_ext = nc.declare_dram_parameter("x_bf", [N, C], BF, isOutput=False)
    wq_ext = nc.declare_dram_parameter("W_qkv_bf", [C, 3 * C], BF, isOutput=False)
    bq_ext = nc.declare_dram_parameter("b_qkv", [3 * C], F32, isOutput=False)
    wo_ext = nc.declare_dram_parameter("W_out_bf", [C, C], BF, isOutput=False)
    bo_ext = nc.declare_dram_parameter("b_out", [C], F32, isOutput=False)
    out_ext = nc.declare_dram_parameter("out", [N, C], BF, isOutput=True)

    with ExitStack() as ctx:
        tc = ctx.enter_context(tile.TileContext(nc, pool_alloc_mode="queue"))
        persist = ctx.enter_context(tc.tile_pool(name="persist", bufs=1))

        # x loads first (HWDGE, bf16 pre-cast on host): they gate the PE
        # transposes that open the kernel. Pool (not persist) so the ring
        # reclaims the space once the transposes have consumed them.
        xbf_stack = ExitStack()
        xbf_pool = xbf_stack.enter_context(tc.tile_pool(name="xbf", bufs=1))
        xbf = []
        for t in range(NT):
            xb = xbf_pool.tile([128, C], BF, tag=f"xb{t}", name=f"xb{t}")
            nc.sync.dma_start(xb[:], x_ext[t * 128:(t + 1) * 128, :])
            xbf.append(xb)

        # identity for PE transposes (gpsimd-built, DVE-fenced)
        ident_g = persist.tile([128, 128], BF, tag="identg")
        masks.make_identity(nc, ident_g[:])
        ident = persist.tile([128, 128], BF, tag="ident")
        nc.vector.tensor_copy(ident[:], ident_g[:])

        # weights: HWDGE loads of host-pre-cast bf16
        wq_sb = []
        for j in range(NCIN):
            w = persist.tile([128, 3 * C], BF, tag=f"wq{j}", name=f"wq{j}")
            nc.sync.dma_start(w[:], wq_ext[j * 128:(j + 1) * 128, :])
            wq_sb.append(w)
        wo_sb = [
            persist.tile([128, C], BF, tag=f"wo{j}", name=f"wo{j}")
            for j in range(NCIN)
        ]

        # biases: per-cout column layout for q/k (f32, read only by DVE) and
        # full-partition broadcast tiles for the V / out-proj DVE adds
        bqT = persist.tile([128, 18], F32, tag="bqT")
        nc.sync.dma_start(bqT[:], bq_ext[:].rearrange("(j p) -> p j", p=128))
        bv_row = persist.tile([1, H, D + 1], F32, tag="bvrow")
        nc.vector.memset(bv_row[:], 0.0)
        nc.sync.dma_start(
            bv_row[:, :, 0:D],
            bq_ext[2 * C:3 * C].rearrange("(a h d) -> a h d", a=1, h=H),
        )
        bo_row = persist.tile([1, C], F32, tag="borow")
        nc.sync.dma_start(bo_row[:], bo_ext[:].rearrange("(a b) -> a b", a=1))
        bv_bc = persist.tile([128, H, D + 1], F32, tag="bvbc")
        nc.gpsimd.partition_broadcast(
            bv_bc[:].rearrange("p h d -> p (h d)"),
            bv_row[:].rearrange("a h d -> a (h d)"),
        )
        bo_bc = persist.tile([128, C], F32, tag="bobc")
        nc.gpsimd.partition_broadcast(bo_bc[:], bo_row[:])

        xT = [persist.tile([128, N], BF, tag=f"xT{j}", name=f"xT{j}") for j in range(NCIN)]
        qk_sb = [persist.tile([128, N], BF, tag=f"qk{j}", name=f"qk{j}") for j in range(2 * NCIN)]
        vaug = [persist.tile([128, H, D + 1], BF, tag=f"v{t}", name=f"v{t}") for t in range(NT)]
        ao = [persist.tile([128, N], BF, tag=f"ao{j}", name=f"ao{j}") for j in range(NCIN)]

        # ---- x transpose: PE transpose of bf16 chunks, DVE copies into xT ----
        with tc.tile_pool(name="pxt", bufs=4, space="PSUM") as pxt_pool:
            for t in range(NT):
                for j in range(NCIN):
                    pxt = pxt_pool.tile([128, 128], BF, tag="pxt")
                    nc.tensor.transpose(
                        pxt[:], xbf[t][:, j * 128:(j + 1) * 128], ident[:]
                    )
                    nc.vector.tensor_copy(
                        xT[j][:, t * 128:(t + 1) * 128], pxt[:]
                    )
        xbf_stack.close()

        # ---- x transpose: PE transpose of bf16 chunks, DVE copies into xT ----
        with tc.tile_pool(name="pxt", bufs=4, space="PSUM") as pxt_pool:
            for t in range(NT):
                for j in range(NCIN):
                    pxt = pxt_pool.tile([128, 128], BF, tag="pxt")
                    nc.tensor.transpose(
                        pxt[:], xbf[t][:, j * 128:(j + 1) * 128], ident[:]
                    )
                    nc.vector.tensor_copy(
                        xT[j][:, t * 128:(t + 1) * 128], pxt[:]
                    )

        # ---- attention pipeline pools ----
        ps_stack = ExitStack()
        ps_pool = ps_stack.enter_context(tc.tile_pool(name="ps", bufs=2, space="PSUM"))
        av_stack = ExitStack()
        pt_pool = ctx.enter_context(tc.tile_pool(name="pt", bufs=34))
        rec_pool = ctx.enter_context(tc.tile_pool(name="rec", bufs=2))
        pbs_pool = ctx.enter_context(tc.tile_pool(name="pbs", bufs=2))
        # scratch slots are single-use: a reused slot would add the shuffle
        # DMA's queue semaphore to the normalize mul's wait list
        scr_pool = ctx.enter_context(tc.tile_pool(name="scr", bufs=2))
        out_pool = ctx.enter_context(tc.tile_pool(name="osb", bufs=2))

        proj_ctx = ExitStack()
        proj_pool = proj_ctx.enter_context(
            tc.tile_pool(name="proj", bufs=2, space="PSUM")
        )

        def _proj_tile():
            return proj_pool.tile([128, N], F32, tag="proj", name="proj")

        def gen_qk_chunk(jout):
            """q/k projection chunk jout (0-5: q, 6-11: k), output transposed.
            Yields after each PE matmul so the driver can interleave."""
            pq = _proj_tile()
            for qc in range(2):
                for jc in range(NCIN):
                    nc.tensor.matmul(
                        pq[:, qc * 512:(qc + 1) * 512],
                        wq_sb[jc][:, jout * 128:(jout + 1) * 128],
                        xT[jc][:, qc * 512:(qc + 1) * 512],
                        start=(jc == 0),
                        stop=(jc == NCIN - 1),
                    )
                    yield
            # PSUM -> SBUF bf16 with per-partition (per-cout) bias add
            nc.vector.tensor_scalar_add(
                qk_sb[jout][:], pq[:], bqT[:, jout:jout + 1]
            )

        def gen_v_chunk(t):
            """V projection for token chunk t, natural layout, into vaug.
            Bias is applied by the DVE drain (no K=1 PE matmul)."""
            pv = _proj_tile()
            for n0, n1 in ((0, 512), (512, 768)):
                for jc in range(NCIN):
                    nc.tensor.matmul(
                        pv[:, n0:n1],
                        xT[jc][:, t * 128:(t + 1) * 128],
                        wq_sb[jc][:, 2 * C + n0:2 * C + n1],
                        start=(jc == 0),
                        stop=(jc == NCIN - 1),
                    )
                    yield
            nc.vector.tensor_add(
                vaug[t][:, :, 0:D],
                pv[:, 0:C].rearrange("p (h d) -> p h d", h=H),
                bv_bc[:, :, 0:D],
            )
            nc.vector.memset(vaug[t][:, :, D:D + 1], 1.0)

        def gen_scores(pj, pts):
            """scores + exp for head pair pj; fills pts[par][kc].
            Both parities are emitted back-to-back per (qc, kc) so the two
            row-tiles (stationary at partitions 0:64 / 64:128) overlap.
            Yields once per kc (4 matmuls + 2 exps)."""
            for kc in range(NT):
                pt0 = pt_pool.tile([128, N], BF, tag="pt")
                pt1 = pt_pool.tile([128, N], BF, tag="pt")
                ps0 = ps_pool.tile([128, N], F32, tag="ps")
                ps1 = ps_pool.tile([128, N], F32, tag="ps")
                for qc in range(2):
                    for par, ps in ((0, ps0), (1, ps1)):
                        base = par * 64
                        nc.tensor.matmul(
                            ps[:, qc * 512:(qc + 1) * 512],
                            qk_sb[NCIN + pj][base:base + 64,
                                             kc * 128:(kc + 1) * 128],
                            qk_sb[pj][base:base + 64, qc * 512:(qc + 1) * 512],
                            start=True,
                            stop=True,
                        )
                nc.scalar.activation(pt0[:], ps0[:], AF.Exp, scale=SCALE)
                nc.scalar.activation(pt1[:], ps1[:], AF.Exp, scale=SCALE)
                pts[0].append(pt0)
                pts[1].append(pt1)
                yield

        av_pool = None

        def _av_norm(pj, av, par, sl, scr):
            # stage the denominator row through SBUF with a regular copy:
            # custom-DVE ops lose their PSUM-read wait in the finalize
            # passes (read-before-matmul-stop on HW; sim can't see it),
            # and DVE in-queue ordering makes copy->recip safe.
            den = rec_pool.tile([1, 512], F32, tag="den", name="den")
            nc.vector.tensor_copy(den[:], av[64:65, :])
            rec = rec_pool.tile([1, 512], F32, tag="rec", name="rec")
            nc.vector.reciprocal_approx_fast(rec[:], den[:])
            pb = pbs_pool.tile([64, 512], F32, tag="pbs", name="pb")
            nc.gpsimd.partition_broadcast(pb[:], rec[:])
            if par == 0:
                nc.vector.tensor_mul(ao[pj][0:64, sl], av[0:64, :], pb[:])
            else:
                nc.vector.tensor_mul(scr[:, sl], av[0:64, :], pb[:])

        def gen_av(pj, pts):
            """AV + normalization for pair pj, qc-outer over pairs of 1-bank
            PSUM tiles: each qc-half normalizes while the other half
            accumulates, so the reciprocal/broadcast/mul latency hides.
            Even head -> ao rows 0:64 directly; odd head -> scratch, DMA
            shuffle to rows 64:128. Yields after each PE matmul."""
            scr = scr_pool.tile([64, N], BF, tag="scr")
            for qc in range(2):
                sl = slice(qc * 512, (qc + 1) * 512)
                av = [
                    av_pool.tile([65, 512], F32, tag=f"av{par}", name=f"av{par}")
                    for par in range(2)
                ]
                for kc in range(NT):
                    for par in range(2):
                        nc.tensor.matmul(
                            av[par][:],
                            vaug[kc][:, 2 * pj + par, :],
                            pts[par][kc][:, sl],
                            start=(kc == 0),
                            stop=(kc == NT - 1),
                        )
                        yield
                for par in range(2):
                    _av_norm(pj, av[par], par, sl, scr)
            nc.sync.dma_start(ao[pj][64:128, :], scr[:])

        def gen_av_kc_outer(pj, pts):
            """kc-outer AV for the final pair: consumes each kc's exp as it
            is emitted (all four PSUM tiles open), so the post-last-exp
            tail is only the last kc's matmuls plus normalization."""
            scr = scr_pool.tile([64, N], BF, tag="scr")
            av = [
                [
                    av_pool.tile([65, 512], F32, tag=f"av{par}", name=f"av{par}")
                    for par in range(2)
                ]
                for qc in range(2)
            ]
            for kc in range(NT):
                for qc in range(2):
                    sl = slice(qc * 512, (qc + 1) * 512)
                    for par in range(2):
                        nc.tensor.matmul(
                            av[qc][par][:],
                            vaug[kc][:, 2 * pj + par, :],
                            pts[par][kc][:, sl],
                            start=(kc == 0),
                            stop=(kc == NT - 1),
                        )
                        yield
            # phased normalize: DVE copies+recips first, then gpsimd
            # broadcasts, then muls — so the DVE never head-of-line
            # blocks on a broadcast, and the exposed post-exp chain is
            # ~4us shorter than four serial copy/recip/bcast/mul chains.
            recs = {}
            for qc in range(2):
                for par in range(2):
                    den = rec_pool.tile([1, 512], F32, tag="den", name="den")
                    nc.vector.tensor_copy(den[:], av[qc][par][64:65, :])
                    rec = rec_pool.tile([1, 512], F32, tag="rec", name="rec")
                    nc.vector.reciprocal_approx_fast(rec[:], den[:])
                    recs[qc, par] = rec
            pbs = {}
            for qc in range(2):
                for par in range(2):
                    pb = pbs_pool.tile([64, 512], F32, tag="pbs", name="pb")
                    nc.gpsimd.partition_broadcast(pb[:], recs[qc, par][:])
                    pbs[qc, par] = pb
            for qc in range(2):
                sl = slice(qc * 512, (qc + 1) * 512)
                nc.vector.tensor_mul(
                    ao[pj][0:64, sl], av[qc][0][0:64, :], pbs[qc, 0][:]
                )
                nc.vector.tensor_mul(scr[:, sl], av[qc][1][0:64, :], pbs[qc, 1][:])
                nc.sync.dma_start(ao[pj][64:128, sl], scr[:, sl])

        def drive(primary, filler, ratio):
            """Alternate: 1 primary step then `ratio` filler steps; drain
            primary; leftover filler is left for the caller."""
            for _ in primary:
                for _ in range(ratio):
                    if next(filler, _SENTINEL) is _SENTINEL:
                        break

        def drain(g):
            for _ in g:
                pass

        def chain(*gens):
            for g in gens:
                yield from g

        # ---- emission schedule ----
        # Phase 1 (PE-bound): all qkv projection work as filler between
        # pairs 0-1's score steps (14 matmuls ≈ 3.2us per step).
        # Phase 2 (exp-paced): pairs 2-5's scores every ~2.2us with all
        # AV work as filler (6 yields per step).
        drain(gen_qk_chunk(0))
        drain(gen_qk_chunk(NCIN + 0))
        pts_all = {pj: [[], []] for pj in range(NPAIR)}
        early_scores = chain(
            gen_scores(0, pts_all[0]), gen_scores(1, pts_all[1])
        )
        filler = chain(
            gen_qk_chunk(1), gen_qk_chunk(NCIN + 1),
            *[gen_v_chunk(t) for t in range(NT)],
            gen_qk_chunk(2), gen_qk_chunk(NCIN + 2),
            gen_qk_chunk(3), gen_qk_chunk(NCIN + 3),
            gen_qk_chunk(4), gen_qk_chunk(NCIN + 4),
            gen_qk_chunk(5), gen_qk_chunk(NCIN + 5),
        )
        drive(early_scores, filler, 14)
        drain(filler)
        proj_ctx.close()
        # W_out is only needed by the out-projection tail: issuing its DMA
        # here keeps the startup bandwidth for x and W_qkv
        for j in range(NCIN):
            nc.sync.dma_start(wo_sb[j][:], wo_ext[j * 128:(j + 1) * 128, :])
        if _DEBUG:
            nc.gpsimd.dma_start(dbg["dbg_xT"][:], xT[0][:])
            nc.gpsimd.dma_start(dbg["dbg_q"][:], qk_sb[0][:])
            nc.gpsimd.dma_start(dbg["dbg_k"][:], qk_sb[NCIN][:])
            nc.gpsimd.dma_start(
                dbg["dbg_v"][:], vaug[0][:].rearrange("p h d -> p (h d)")
            )
            nc.gpsimd.dma_start(dbg["dbg_pt"][:], pts_all[0][0][0][:])

        av_pool = av_stack.enter_context(
            tc.tile_pool(name="av", bufs=2, space="PSUM")
        )

        late_scores = chain(
            gen_scores(2, pts_all[2]), gen_scores(3, pts_all[3]),
            gen_scores(4, pts_all[4]), gen_scores(5, pts_all[5]),
        )
        av_chain = chain(
            *[gen_av(pj, pts_all[pj]) for pj in range(NPAIR - 1)],
            gen_av_kc_outer(NPAIR - 1, pts_all[NPAIR - 1]),
        )
        drive(late_scores, av_chain, 6)
        drain(av_chain)
        av_stack.close()
        ps_stack.close()
        if _DEBUG:
            nc.gpsimd.dma_start(dbg["dbg_ao"][:], ao[0][:])

        # ---- output projection (bias via DVE add on the PSUM drain) ----
        # Two waves of 4 token chunks; within a wave all jc 0-4 partials
        # are emitted first so they overlap the final pair's normalize
        # chain, and only the jc=5 closers wait on ao[5].
        pf_pool = ctx.enter_context(tc.tile_pool(name="pf", bufs=4, space="PSUM"))
        for wave in range(2):
            ts = range(wave * 4, wave * 4 + 4)
            pfs = {}
            for t in ts:
                pfs[t] = pf_pool.tile([128, N], F32, tag="pf", name="pf")
                for n0, n1 in ((0, 512), (512, 768)):
                    for jc in range(NCIN - 1):
                        nc.tensor.matmul(
                            pfs[t][:, n0:n1],
                            ao[jc][:, t * 128:(t + 1) * 128],
                            wo_sb[jc][:, n0:n1],
                            start=(jc == 0),
                            stop=False,
                        )
            for t in ts:
                for n0, n1 in ((0, 512), (512, 768)):
                    nc.tensor.matmul(
                        pfs[t][:, n0:n1],
                        ao[NCIN - 1][:, t * 128:(t + 1) * 128],
                        wo_sb[NCIN - 1][:, n0:n1],
                        start=False,
                        stop=True,
                    )
                osb = out_pool.tile([128, C], BF, tag="osb")
                nc.vector.tensor_add(osb[:], pfs[t][:, 0:C], bo_bc[:])
                nc.sync.dma_start(out_ext[t * 128:(t + 1) * 128, :], osb[:])

    nc.finalize()
    return nc


_NC = None


def _get_nc():
    global _NC
    if _NC is None:
        _NC = build()
    return _NC


def _run(inputs, trace=False, **kw):
    from concourse.bass_utils import run_bass_kernel_spmd

    import ml_dtypes

    nc = _get_nc()
    bf = ml_dtypes.bfloat16
    x = np.ascontiguousarray(np.asarray(inputs["x"], dtype=np.float32).astype(bf))
    shared = {
        "W_qkv_bf": np.ascontiguousarray(
            np.asarray(inputs["W_qkv"], np.float32).astype(bf)),
        "b_qkv": np.ascontiguousarray(np.asarray(inputs["b_qkv"], np.float32)),
        "W_out_bf": np.ascontiguousarray(
            np.asarray(inputs["W_out"], np.float32).astype(bf)),
        "b_out": np.ascontiguousarray(np.asarray(inputs["b_out"], np.float32)),
    }
    in_maps = [dict(shared, x_bf=x[c]) for c in range(NCORES)]
    res = run_bass_kernel_spmd(
        nc, in_maps, core_ids=list(range(NCORES)), trace=trace, **kw
    )
    out = np.stack([res.results[c]["out"] for c in range(NCORES)], axis=0)
    return out.astype(np.float32), res


def kernel(**inputs):
    out, _ = _run(inputs, trace=False)
    return out


# revision 44
# speedup vs baseline: 1.0340x; 1.0269x over previous
"""Multi-head attention forward (B=8, N=1024, C=768, H=12, D=64) on 8 TRN2 NeuronCores.

Strategy: pure data-parallel over batch (batch 8 == 8 cores, no collectives).
Each core computes one full batch element. Host scatters inputs / gathers outputs.

Per-core kernel (bf16 TensorE compute, f32 PSUM accumulation):
  xT   = transpose(x)                          PE transposes, [C, N]
  qkT  = W_qkv[:, :2C].T @ xT (+b)             [2C, N]  (q,k transposed: head dim on partitions)
  V    = x @ W_qkv[:, 2C:] (+b)                [N, C]   (natural: k-token on partitions)
  PT_h = exp(SCALE * kT_h.T @ qT_h)            [N_k, N_q] per head (scores transposed; no
                                               max-subtraction needed: scores ~ N(0,1))
  av_h = [v_h | 1].T @ PT_h                    [65, N_q]: rows 0-63 unnormalized out^T,
                                               row 64 = softmax denominator
  aoT_h = av_h[0:64] * bcast(1/av_h[64])       attn_out transposed [C, N]
  out  = aoT.T @ W_out + b                     [N, C]

Performance notes (309us baseline -> 217us):
 - inputs are pre-cast to bf16 on the host and loaded with HWDGE
   (sync.dma_start): kills the serial ~650ns-per-DMA gpsimd issue chain
   that gated startup, halves input DMA bytes. First exp at ~24us
   (was ~38us).
 - score matmuls have K=64 (head dim): bass auto-emits them row-tiled
   64x128, so the two parities of a head pair (stationary at partitions
   0:64 / 64:128) execute on independent PE tiles T0/T8. Emitting both
   parities back-to-back per (qc, kc) lets the tiles overlap.
 - AV runs qc-outer over 1-bank [65, 512] PSUM tiles (2 tags x bufs=2)
   so each qc-half normalizes while the other accumulates; the final
   pair runs kc-outer so the post-last-exp tail is only the last kc.
 - softmax denominators: PSUM row staged to SBUF with a regular
   tensor_copy, then reciprocal_approx_fast (~5x faster than
   InstReciprocal; 18 bits, plenty above bf16). The copy is mandatory
   for correctness, not just speed: custom-DVE ops lose their
   PSUM-read wait in the finalize passes and race the accumulating
   matmuls on HW (CoreSim orders by program, so only HW corrupts).
 - V / out-proj biases are DVE tensor_adds against partition-broadcast
   bias tiles instead of K=1 PE matmuls.
 - exp (ScalarE, ~106us busy) paces phase 2 at ~2.2us per kc step;
   phase 1 is PE-bound (qkv projection work interleaved with pairs
   0-1's scores at ratio 14), phase 2 interleaves pairs 2-5's scores
   with all AV work at ratio 6.
 - PSUM budget (8 banks): phase 1 = scores 4 + proj 4; phase 2 =
   scores 4 + AV 4; tail = out-proj 4x2-bank tiles in two waves whose
   jc0-4 partials overlap the final normalize chain.
"""
import sys

sys.path.insert(0, "/opt/trn_rl_repo")

from contextlib import ExitStack

import numpy as np

import concourse.bass as bass
import concourse.bacc as bacc
import concourse.tile as tile
from concourse import masks, mybir

_SENTINEL = object()
F32 = mybir.dt.float32
BF = mybir.dt.bfloat16
AF = mybir.ActivationFunctionType

B, N, C, H, D = 8, 1024, 768, 12, 64
SCALE = D ** -0.5
NCORES = 8
NT = N // 128      # 8 token chunks
NCIN = C // 128    # 6 input-channel chunks
NPAIR = H // 2     # 6 head pairs

_DEBUG = False


def build():
    nc = bacc.Bacc()
    if _DEBUG:
        dbg = {
            nm: nc.declare_dram_parameter(nm, shape, F32, isOutput=True)
            for nm, shape in [
                ("dbg_xT", [128, N]), ("dbg_q", [128, N]), ("dbg_k", [128, N]),
                ("dbg_v", [128, H * (D + 1)]), ("dbg_pt", [128, N]),
                ("dbg_ao", [128, N]),
            ]
        }
    x_ext = nc.declare_dram_parameter("x_bf", [N, C], BF, isOutput=False)
    wq_ext = nc.declare_dram_parameter("W_qkv_bf", [C, 3 * C], BF, isOutput=False)
    bq_ext = nc.declare_dram_parameter("b_qkv", [3 * C], F32, isOutput=False)
    wo_ext = nc.declare_dram_parameter("W_out_bf", [C, C], BF, isOutput=False)
    bo_ext = nc.declare_dram_parameter("b_out", [C], F32, isOutput=False)
    out_ext = nc.declare_dram_parameter("out", [N, C], F32, isOutput=True)

    with ExitStack() as ctx:
        tc = ctx.enter_context(tile.TileContext(nc, pool_alloc_mode="queue"))
        persist = ctx.enter_context(tc.tile_pool(name="persist", bufs=1))

        # x loads first (HWDGE, bf16 pre-cast on host): they gate the PE
        # transposes that open the kernel. Pool (not persist) so the ring
        # reclaims the space once the transposes have consumed them.
        xbf_stack = ExitStack()
        xbf_pool = xbf_stack.enter_context(tc.tile_pool(name="xbf", bufs=1))
        xbf = []
        for t in range(NT):
            xb = xbf_pool.tile([128, C], BF, tag=f"xb{t}", name=f"xb{t}")
            nc.sync.dma_start(xb[:], x_ext[t * 128:(t + 1) * 128, :])
            xbf.append(xb)

        # identity for PE transposes (gpsimd-built, DVE-fenced)
        ident_g = persist.tile([128, 128], BF, tag="identg")
        masks.make_identity(nc, ident_g[:])
        ident = persist.tile([128, 128], BF, tag="ident")
        nc.vector.tensor_copy(ident[:], ident_g[:])

        # weights: HWDGE loads of host-pre-cast bf16
        wq_sb = []
        for j in range(NCIN):
            w = persist.tile([128, 3 * C], BF, tag=f"wq{j}", name=f"wq{j}")
            nc.sync.dma_start(w[:], wq_ext[j * 128:(j + 1) * 128, :])
            wq_sb.append(w)
        wo_sb = [
            persist.tile([128, C], BF, tag=f"wo{j}", name=f"wo{j}")
            for j in range(NCIN)
        ]

        # biases: per-cout column layout for q/k (f32, read only by DVE) and
        # full-partition broadcast tiles for the V / out-proj DVE adds
        bqT = persist.tile([128, 18], F32, tag="bqT")
        nc.sync.dma_start(bqT[:], bq_ext[:].rearrange("(j p) -> p j", p=128))
        bv_row = persist.tile([1, H, D + 1], F32, tag="bvrow")
        nc.vector.memset(bv_row[:], 0.0)
        nc.sync.dma_start(
            bv_row[:, :, 0:D],
            bq_ext[2 * C:3 * C].rearrange("(a h d) -> a h d", a=1, h=H),
        )
        bo_row = persist.tile([1, C], F32, tag="borow")
        nc.sync.dma_start(bo_row[:], bo_ext[:].rearrange("(a b) -> a b", a=1))
        bv_bc = persist.tile([128, H, D + 1], F32, tag="bvbc")
        nc.gpsimd.partition_broadcast(
            bv_bc[:].rearrange("p h d -> p (h d)"),
            bv_row[:].rearrange("a h d -> a (h d)"),
        )
        bo_bc = persist.tile([128, C], F32, tag="bobc")
        nc.gpsimd.partition_broadcast(bo_bc[:], bo_row[:])

        xT = [persist.tile([128, N], BF, tag=f"xT{j}", name=f"xT{j}") for j in range(NCIN)]
        qk_sb = [persist.tile([128, N], BF, tag=f"qk{j}", name=f"qk{j}") for j in range(2 * NCIN)]
        vaug = [persist.tile([128, H, D + 1], BF, tag=f"v{t}", name=f"v{t}") for t in range(NT)]
        ao = [persist.tile([128, N], BF, tag=f"ao{j}", name=f"ao{j}") for j in range(NCIN)]

        # ---- x transpose: PE transpose of bf16 chunks, DVE copies into xT ----
        with tc.tile_pool(name="pxt", bufs=4, space="PSUM") as pxt_pool:
            for t in range(NT):
                for j in range(NCIN):
                    pxt = pxt_pool.tile([128, 128], BF, tag="pxt")
                    nc.tensor.transpose(
                        pxt[:], xbf[t][:, j * 128:(j + 1) * 128], ident[:]
                    )
                    nc.vector.tensor_copy(
                        xT[j][:, t * 128:(t + 1) * 128], pxt[:]
                    )
        xbf_stack.close()

        # ---- x transpose: PE transpose of bf16 chunks, DVE copies into xT ----
        with tc.tile_pool(name="pxt", bufs=4, space="PSUM") as pxt_pool:
            for t in range(NT):
                for j in range(NCIN):
                    pxt = pxt_pool.tile([128, 128], BF, tag="pxt")
                    nc.tensor.transpose(
                        pxt[:], xbf[t][:, j * 128:(j + 1) * 128], ident[:]
                    )
                    nc.vector.tensor_copy(
                        xT[j][:, t * 128:(t + 1) * 128], pxt[:]
                    )
        xbf_stack.close()

        # ---- attention pipeline pools ----
        ps_stack = ExitStack()
        ps_pool = ps_stack.enter_context(tc.tile_pool(name="ps", bufs=2, space="PSUM"))
        av_stack = ExitStack()
        pt_pool = ctx.enter_context(tc.tile_pool(name="pt", bufs=34))
        rec_pool = ctx.enter_context(tc.tile_pool(name="rec", bufs=2))
        pbs_pool = ctx.enter_context(tc.tile_pool(name="pbs", bufs=2))
        # scratch slots are single-use: a reused slot would add the shuffle
        # DMA's queue semaphore to the normalize mul's wait list
        scr_pool = ctx.enter_context(tc.tile_pool(name="scr", bufs=2))
        out_pool = ctx.enter_context(tc.tile_pool(name="osb", bufs=2))

        proj_ctx = ExitStack()
        proj_pool = proj_ctx.enter_context(
            tc.tile_pool(name="proj", bufs=2, space="PSUM")
        )

        def _proj_tile():
            return proj_pool.tile([128, N], F32, tag="proj", name="proj")

        def gen_qk_chunk(jout):
            """q/k projection chunk jout (0-5: q, 6-11: k), output transposed.
            Yields after each PE matmul so the driver can interleave."""
            pq = _proj_tile()
            for qc in range(2):
                for jc in range(NCIN):
                    nc.tensor.matmul(
                        pq[:, qc * 512:(qc + 1) * 512],
                        wq_sb[jc][:, jout * 128:(jout + 1) * 128],
                        xT[jc][:, qc * 512:(qc + 1) * 512],
                        start=(jc == 0),
                        stop=(jc == NCIN - 1),
                    )
                    yield
            # PSUM -> SBUF bf16 with per-partition (per-cout) bias add
            nc.vector.tensor_scalar_add(
                qk_sb[jout][:], pq[:], bqT[:, jout:jout + 1]
            )

        def gen_v_chunk(t):
            """V projection for token chunk t, natural layout, into vaug.
            Bias is applied by the DVE drain (no K=1 PE matmul)."""
            pv = _proj_tile()
            for n0, n1 in ((0, 512), (512, 768)):
                for jc in range(NCIN):
                    nc.tensor.matmul(
                        pv[:, n0:n1],
                        xT[jc][:, t * 128:(t + 1) * 128],
                        wq_sb[jc][:, 2 * C + n0:2 * C + n1],
                        start=(jc == 0),
                        stop=(jc == NCIN - 1),
                    )
                    yield
            nc.vector.tensor_add(
                vaug[t][:, :, 0:D],
                pv[:, 0:C].rearrange("p (h d) -> p h d", h=H),
                bv_bc[:, :, 0:D],
            )
            nc.vector.memset(vaug[t][:, :, D:D + 1], 1.0)

        def gen_scores(pj, pts):
            """scores + exp for head pair pj; fills pts[par][kc].
            Both parities are emitted back-to-back per (qc, kc) so the two
            row-tiles (stationary at partitions 0:64 / 64:128) overlap.
            Yields once per kc (4 matmuls + 2 exps)."""
            for kc in range(NT):
                pt0 = pt_pool.tile([128, N], BF, tag="pt")
                pt1 = pt_pool.tile([128, N], BF, tag="pt")
                ps0 = ps_pool.tile([128, N], F32, tag="ps")
                ps1 = ps_pool.tile([128, N], F32, tag="ps")
                for qc in range(2):
                    for par, ps in ((0, ps0), (1, ps1)):
                        base = par * 64
                        nc.tensor.matmul(
                            ps[:, qc * 512:(qc + 1) * 512],
                            qk_sb[NCIN + pj][base:base + 64,
                                             kc * 128:(kc + 1) * 128],
                            qk_sb[pj][base:base + 64, qc * 512:(qc + 1) * 512],
                            start=True,
                            stop=True,
                        )
                nc.scalar.activation(pt0[:], ps0[:], AF.Exp, scale=SCALE)
                nc.scalar.activation(pt1[:], ps1[:], AF.Exp, scale=SCALE)
                pts[0].append(pt0)
                pts[1].append(pt1)
                yield

        av_pool = None

        def _av_norm(pj, av, par, sl, scr):
            # stage the denominator row through SBUF with a regular copy:
            # custom-DVE ops lose their PSUM-read wait in the finalize
            # passes (read-before-matmul-stop on HW; sim can't see it),
            # and DVE in-queue ordering makes copy->recip safe.
            den = rec_pool.tile([1, 512], F32, tag="den", name="den")
            nc.vector.tensor_copy(den[:], av[64:65, :])
            rec = rec_pool.tile([1, 512], F32, tag="rec", name="rec")
            nc.vector.reciprocal_approx_fast(rec[:], den[:])
            pb = pbs_pool.tile([64, 512], F32, tag="pbs", name="pb")
            nc.gpsimd.partition_broadcast(pb[:], rec[:])
            if par == 0:
                nc.vector.tensor_mul(ao[pj][0:64, sl], av[0:64, :], pb[:])
            else:
                nc.vector.tensor_mul(scr[:, sl], av[0:64, :], pb[:])

        def gen_av(pj, pts):
            """AV + normalization for pair pj, qc-outer over pairs of 1-bank
            PSUM tiles: each qc-half normalizes while the other half
            accumulates, so the reciprocal/broadcast/mul latency hides.
            Even head -> ao rows 0:64 directly; odd head -> scratch, DMA
            shuffle to rows 64:128. Yields after each PE matmul."""
            scr = scr_pool.tile([64, N], BF, tag="scr")
            for qc in range(2):
                sl = slice(qc * 512, (qc + 1) * 512)
                av = [
                    av_pool.tile([65, 512], F32, tag=f"av{par}", name=f"av{par}")
                    for par in range(2)
                ]
                for kc in range(NT):
                    for par in range(2):
                        nc.tensor.matmul(
                            av[par][:],
                            vaug[kc][:, 2 * pj + par, :],
                            pts[par][kc][:, sl],
                            start=(kc == 0),
                            stop=(kc == NT - 1),
                        )
                        yield
                for par in range(2):
                    _av_norm(pj, av[par], par, sl, scr)
            nc.sync.dma_start(ao[pj][64:128, :], scr[:])

        def gen_av_kc_outer(pj, pts):
            """kc-outer AV for the final pair: consumes each kc's exp as it
            is emitted (all four PSUM tiles open), so the post-last-exp
            tail is only the last kc's matmuls plus normalization."""
            scr = scr_pool.tile([64, N], BF, tag="scr")
            av = [
                [
                    av_pool.tile([65, 512], F32, tag=f"av{par}", name=f"av{par}")
                    for par in range(2)
                ]
                for qc in range(2)
            ]
            for kc in range(NT):
                for qc in range(2):
                    sl = slice(qc * 512, (qc + 1) * 512)
                    for par in range(2):
                        nc.tensor.matmul(
                            av[qc][par][:],
                            vaug[kc][:, 2 * pj + par, :],
                            pts[par][kc][:, sl],
                            start=(kc == 0),
                            stop=(kc == NT - 1),
                        )
                        yield
            # phased normalize: DVE copies+recips first, then gpsimd
            # broadcasts, then muls — so the DVE never head-of-line
            # blocks on a broadcast, and the exposed post-exp chain is
            # ~4us shorter than four serial copy/recip/bcast/mul chains.
            recs = {}
            for qc in range(2):
                for par in range(2):
                    den = rec_pool.tile([1, 512], F32, tag="den", name="den")
                    nc.vector.tensor_copy(den[:], av[qc][par][64:65, :])
                    rec = rec_pool.tile([1, 512], F32, tag="rec", name="rec")
                    nc.vector.reciprocal_approx_fast(rec[:], den[:])
                    recs[qc, par] = rec
            pbs = {}
            for qc in range(2):
                for par in range(2):
                    pb = pbs_pool.tile([64, 512], F32, tag="pbs", name="pb")
                    nc.gpsimd.partition_broadcast(pb[:], recs[qc, par][:])
                    pbs[qc, par] = pb
            for qc in range(2):
                sl = slice(qc * 512, (qc + 1) * 512)
                nc.vector.tensor_mul(
                    ao[pj][0:64, sl], av[qc][0][0:64, :], pbs[qc, 0][:]
                )
                nc.vector.tensor_mul(scr[:, sl], av[qc][1][0:64, :], pbs[qc, 1][:])
                nc.sync.dma_start(ao[pj][64:128, sl], scr[:, sl])

        def drive(primary, filler, ratio):
            """Alternate: 1 primary step then `ratio` filler steps; drain
            primary; leftover filler is left for the caller."""
            for _ in primary:
                for _ in range(ratio):
                    if next(filler, _SENTINEL) is _SENTINEL:
                        break

        def drain(g):
            for _ in g:
                pass

        def chain(*gens):
            for g in gens:
                yield from g

        # ---- emission schedule ----
        # Phase 1 (PE-bound): all qkv projection work as filler between
        # pairs 0-1's score steps (14 matmuls ≈ 3.2us per step).
        # Phase 2 (exp-paced): pairs 2-5's scores every ~2.2us with all
        # AV work as filler (6 yields per step).
        drain(gen_qk_chunk(0))
        drain(gen_qk_chunk(NCIN + 0))
        pts_all = {pj: [[], []] for pj in range(NPAIR)}
        early_scores = chain(
            gen_scores(0, pts_all[0]), gen_scores(1, pts_all[1])
        )
        filler = chain(
            gen_qk_chunk(1), gen_qk_chunk(NCIN + 1),
            *[gen_v_chunk(t) for t in range(NT)],
            gen_qk_chunk(2), gen_qk_chunk(NCIN + 2),
            gen_qk_chunk(3), gen_qk_chunk(NCIN + 3),
            gen_qk_chunk(4), gen_qk_chunk(NCIN + 4),
            gen_qk_chunk(5), gen_qk_chunk(NCIN + 5),
        )
        drive(early_scores, filler, 14)
        drain(filler)
        proj_ctx.close()
        # W_out is only needed by the out-projection tail: issuing its DMA
        # here keeps the startup bandwidth for x and W_qkv
        for j in range(NCIN):
            nc.sync.dma_start(wo_sb[j][:], wo_ext[j * 128:(j + 1) * 128, :])
        if _DEBUG:
            nc.gpsimd.dma_start(dbg["dbg_xT"][:], xT[0][:])
            nc.gpsimd.dma_start(dbg["dbg_q"][:], qk_sb[0][:])
            nc.gpsimd.dma_start(dbg["dbg_k"][:], qk_sb[NCIN][:])
            nc.gpsimd.dma_start(
                dbg["dbg_v"][:], vaug[0][:].rearrange("p h d -> p (h d)")
            )
            nc.gpsimd.dma_start(dbg["dbg_pt"][:], pts_all[0][0][0][:])

        av_pool = av_stack.enter_context(
            tc.tile_pool(name="av", bufs=2, space="PSUM")
        )

        late_scores = chain(
            gen_scores(2, pts_all[2]), gen_scores(3, pts_all[3]),
            gen_scores(4, pts_all[4]), gen_scores(5, pts_all[5]),
        )
        av_chain = chain(
            *[gen_av(pj, pts_all[pj]) for pj in range(NPAIR - 1)],
            gen_av_kc_outer(NPAIR - 1, pts_all[NPAIR - 1]),
        )
        drive(late_scores, av_chain, 6)
        drain(av_chain)
        av_stack.close()
        ps_stack.close()
        if _DEBUG:
            nc.gpsimd.dma_start(dbg["dbg_ao"][:], ao[0][:])

        # ---- output projection (bias via DVE add on the PSUM drain) ----
        # Two waves of 4 token chunks; within a wave all jc 0-4 partials
        # are emitted first so they overlap the final pair's normalize
        # chain, and only the jc=5 closers wait on ao[5].
        pf_pool = ctx.enter_context(tc.tile_pool(name="pf", bufs=4, space="PSUM"))
        for wave in range(2):
            ts = range(wave * 4, wave * 4 + 4)
            pfs = {}
            for t in ts:
                pfs[t] = pf_pool.tile([128, N], F32, tag="pf", name="pf")
                for n0, n1 in ((0, 512), (512, 768)):
                    for jc in range(NCIN - 1):
                        nc.tensor.matmul(
                            pfs[t][:, n0:n1],
                            ao[jc][:, t * 128:(t + 1) * 128],
                            wo_sb[jc][:, n0:n1],
                            start=(jc == 0),
                            stop=False,
                        )
            for t in ts:
                for n0, n1 in ((0, 512), (512, 768)):
                    nc.tensor.matmul(
                        pfs[t][:, n0:n1],
                        ao[NCIN - 1][:, t * 128:(t + 1) * 128],
                        wo_sb[NCIN - 1][:, n0:n1],
                        start=False,
                        stop=True,
                    )
                osb = out_pool.tile([128, C], F32, tag="osb")
                nc.vector.tensor_add(osb[:], pfs[t][:, 0:C], bo_bc[:])
                nc.sync.dma_start(out_ext[t * 128:(t + 1) * 128, :], osb[:])

    nc.finalize()
    return nc


_NC = None


def _get_nc():
    global _NC
    if _NC is None:
        _NC = build()
    return _NC


def _run(inputs, trace=False, **kw):
    from concourse.bass_utils import run_bass_kernel_spmd

    import ml_dtypes

    nc = _get_nc()
    bf = ml_dtypes.bfloat16
    x = np.ascontiguousarray(np.asarray(inputs["x"], dtype=np.float32).astype(bf))
    shared = {
        "W_qkv_bf": np.ascontiguousarray(
            np.asarray(inputs["W_qkv"], np.float32).astype(bf)),
        "b_qkv": np.ascontiguousarray(np.asarray(inputs["b_qkv"], np.float32)),
        "W_out_bf": np.ascontiguousarray(
            np.asarray(inputs["W_out"], np.float32).astype(bf)),
        "b_out": np.ascontiguousarray(np.asarray(inputs["b_out"], np.float32)),
    }
    in_maps = [dict(shared, x_bf=x[c]) for c in range(NCORES)]
    res = run_bass_kernel_spmd(
        nc, in_maps, core_ids=list(range(NCORES)), trace=trace, **kw
    )
    out = np.stack([res.results[c]["out"] for c in range(NCORES)], axis=0)
    return out.astype(np.float32), res


def kernel(**inputs):
    out, _ = _run(inputs, trace=False)
    return out
